# revision 21
# baseline (speedup 1.0000x reference)
"""Trainium2 Bass kernel for AttnNoProjVal.

Per batch element b (one NeuronCore each, B=8), using the identity
  scores = q k^T = hs M hs^T + (hs u) 1^T + 1 (hs v)^T + bk.bq,
  M = Wk^T Wq (host-folded), u = Wk^T bq, v = Wq^T bk:
the v and constant terms are per-QUERY-column offsets, which cancel exactly
in softmax and are dropped; the u term is a per-KEY offset folded into the
exp bias. The kernel computes one fused projection g^T = M^T hs^T, then
  scoresT[kp,qp] = (g^T)[:,kp] . (hsq^T)[:,qp]
  E = exp(scoresT/32 + bias[kp])    bias = (hs u)/32 - 3 + mask (host-prep)
  out[qp,:] = (E^T hsbk) / den,  den via esum = sum_k E[k] on DVE then a
  single [128,1] ones-matmul per 128-query group.

vs the v1 fp16 kernel (trace-driven, NTFF analysis; v1 347us -> 280us
traced at warm clock, run-to-run P0 power-downclock adds ~18%):
- the denominator's 240 one-column matmuls (~32ns NX floor each, ~7.6us of
  PE time) are replaced by DVE chunk-adds (idle engine) + 16 tiny matmuls.
- phase-A start: sync queue carries m g0 then hstk b0..b4, scalar queue m
  g1..g7 then hstk b5/b6; first key blocks shrunk to 64/64/128 wide and the
  (block, oc) chains emitted in a wavefront matching DMA arrival order.
  PE gaps in phase A dropped from ~7.4us (v2: sync's big late hstk blocks +
  early q0/q1 ate HBM while m g6/g7 crawled at 11-50GB/s) to ~1.4us.
- 8 warm-up matmuls on stride-0 broadcasts of the const-pool bf16 1.0
  scalar (memset by the NEFF preamble BEFORE the kernel-entry barrier, so
  the PE starts at ~7.3us with zero in-kernel deps) pre-warm the HAM clock
  gate (cold PE runs at 1.2GHz for the first ~3.4us of activity); real
  chains then start at full rate the moment their DMA data lands.
- scores PSUM pool allocated in the outer scope on banks phase-A never
  touches: allocating it inside phase B serialized its first tile behind
  all 56 phase-A PSUM->SBUF casts (pool-boundary wait, 650ns PE gap).
  Phase-A PSUM shrinks to 5 bufs (still ahead of the cast turnaround).
- out is stored fp16 (2e-2 rel budget, adds ~5e-4) halving store traffic;
  each 512-col half is scaled and stored by a different engine/queue
  (vector+sync / scalar+scalar) and the last group's po0/po1 chains are
  de-interleaved, shortening the serial tail.
- phase-B loads stay off the phase-A HBM window: hsbk queues behind the m
  groups on the scalar queue (pinned at 25% PE progress); q0/q1 go on the
  gpsimd software queue pinned at 45%/65% via dummy-copy WAW deps (the tile
  scheduler hoists dependency-free DMA triggers to the engine-stream front,
  so emission order alone cannot delay them).
- fp8/DoubleRow rejected by measurement: e4m3's 3-bit mantissa on E or V
  alone gives 3.2e-2 max rel err (budget 2e-2; host-simulated), and
  residual-pair schemes cost >= fp16. Matmul moving dim >512 is blocked by
  the one-PSUM-bank rule, so 512-row fp16 streams are the PE floor
  (measured: 216ns/512-row spacing warm = the documented roofline; LDW
  fully hidden). Remaining fixed overhead: ~7us NEFF preamble + ~10us
  runtime semaphore-zero teardown, not kernel-controllable.
"""

import sys

sys.path.insert(0, "/opt/trn_rl_repo")

from contextlib import ExitStack

import numpy as np

import concourse.tile as tile
from concourse import bacc, mybir
from concourse.bass_utils import run_bass_kernel_spmd

B, S, H = 8, 2048, 1024
N_CORES = 8
HC = H // 128   # 8 chunks of the hidden dim
QB = S // 512   # 4 query blocks
F32 = mybir.dt.float32
F16 = mybir.dt.float16

NKC_DEFAULT = 15  # key chunks after mask compaction (padded to 128)

# m in per-oc column blocks, h-major inside each group
MG = [(i * 128, 128) for i in range(HC)]  # (start, width)
MG_OFS = [0]
for _s, _w in MG:
    MG_OFS.append(MG_OFS[-1] + HC * _w)

_CACHED_NC = {}


def _kb_blocks(nk):
    """key-block (offset, width) list for the projection moving dim; small
    leading blocks (64/64/128/256) shrink the first chain's DMA dependency
    so the PE starts as soon as m g0 + 64 keys have landed."""
    kb = []
    o = 0
    for w in (64, 64, 128, 256):
        if o < nk and nk - o >= w:
            kb.append((o, w))
            o += w
    while o < nk:
        w = min(512, nk - o)
        kb.append((o, w))
        o += w
    return kb


def _kb_split(kb):
    """hstk blocks: all but the last two stream on sync (after m g0); the
    last two queue on scalar behind the m groups, so the sync queue's big
    late blocks can't starve the m stream mid-phase (measured: sync ran at
    319GB/s on b4-b6 at t=16-19us while scalar's m g6/g7 crawled at
    11-50GB/s, stalling the PE 7.4us)."""
    n_sync = max(1, len(kb) - 2)
    return n_sync


def _phase_a_order(kb):
    """(block, oc) chain emission order matching expected DMA arrival:
    sync = m g0, hstk b0..b[n-3]; scalar = m g1..g7, hstk b[n-2], b[n-1].
    Times in us relative to sync-queue first data; scalar queue observed
    ~1.4us later."""
    n_sync = _kb_split(kb)
    m_t = [0.0] * HC
    m_t[0] = 1.31  # g0 leads the sync queue
    for g in range(1, HC):
        m_t[g] = 1.4 + 1.31 * g
    k_t = [0.0] * len(kb)
    cum = 1.31  # after m g0 on sync
    for j in range(n_sync):
        cum += (kb[j][1] * 2048) / 190e3
        k_t[j] = cum
    cum = m_t[HC - 1]  # scalar: after the last m group
    for j in range(n_sync, len(kb)):
        cum += (kb[j][1] * 2048) / 190e3
        k_t[j] = cum
    pairs = [(j, g) for j in range(len(kb)) for g in range(HC)]
    pairs.sort(key=lambda p: (max(m_t[p[1]], k_t[p[0]]), p[0], p[1]))
    return pairs


def build_nc(nkc=NKC_DEFAULT):
    nk = nkc * 128
    kb = _kb_blocks(nk)
    nc = bacc.Bacc(None, target_bir_lowering=False)

    # all inputs host-relaid to [128 partitions, X] with contiguous rows
    hstq = nc.dram_tensor("hstq", [128, QB * HC * 512], F16, kind="ExternalInput")
    hstk = nc.dram_tensor("hstk", [128, HC * nk], F16, kind="ExternalInput")
    hsbk = nc.dram_tensor("hsbk", [128, nkc * H], F16, kind="ExternalInput")
    mt = nc.dram_tensor("mt", [128, HC * H], F16, kind="ExternalInput")
    mk = nc.dram_tensor("mk", [128, nkc], F32, kind="ExternalInput")
    out = nc.dram_tensor("out", [S, H], F16, kind="ExternalOutput")

    with tile.TileContext(nc) as tc, ExitStack() as whole:
        singles = whole.enter_context(tc.tile_pool(name="singles", bufs=1))
        gt_pool = whole.enter_context(tc.tile_pool(name="gtp", bufs=1))
        hsbk_pool = whole.enter_context(tc.tile_pool(name="hsbkp", bufs=1))
        qcol_pool = whole.enter_context(tc.tile_pool(name="qcolp", bufs=2))
        # scores PSUM lives in the outer scope on banks psA never touches:
        # allocating it inside phase B would serialize its first tile behind
        # ALL phase-A CASTs (pool-boundary wait, measured 650ns PE gap)
        ps_s = whole.enter_context(tc.tile_pool(name="pss", bufs=3, space="PSUM"))

        bias_sb = singles.tile([128, nkc], F32, tag="bias", name="bias_sb")
        ones_sb = singles.tile([128, 1], F16, tag="ones", name="ones_sb")
        nc.vector.memset(ones_sb[:], 1.0)

        # g^T = M^T hs^T over compacted keys; resident for the whole kernel
        gt = [gt_pool.tile([128, nk], F16, tag=f"gt{d}", name=f"gt{d}") for d in range(HC)]
        hsbk_sb = hsbk_pool.tile([128, nkc * H], F16, tag="hsbk", name="hsbk_sb")

        # ---- Phase A: fused projection g^T into SBUF.
        with ExitStack() as pa:
            wt_pool = pa.enter_context(tc.tile_pool(name="wtp", bufs=1))
            psA = pa.enter_context(tc.tile_pool(name="psA", bufs=5, space="PSUM"))

            m_sb = wt_pool.tile([128, HC * H], F16, tag="m", name="m_sb")
            hstk_sb = wt_pool.tile([128, HC * nk], F16, tag="hstk", name="hstk_sb")

            # HAM pre-warm: the PE clock gate needs ~3.4us of sustained
            # activity to release (1.2 -> full clock). Operands are stride-0
            # broadcasts of the const-pool bf16 1.0 scalar, which the NEFF
            # preamble memsets BEFORE the kernel-entry barrier — so the first
            # LDWEIGHTS has no in-kernel dependency and the PE starts ~1us
            # earlier than any same-kernel memset allows. The product is
            # never read (next pool tile overwrites with start=True).
            warm_lhs = nc.const_aps.tensor(1.0, (128, 128), mybir.dt.bfloat16)
            warm_rhs = nc.const_aps.tensor(1.0, (128, 512), mybir.dt.bfloat16)
            # 10 dummies: at warm clock they end ~9.4us, keeping the PE idle
            # before the first data-gated chain (~12us) under the 3.4us HAM
            # MID re-throttle window; at cold (427ns each) they end ~11.5us,
            # still hidden behind the DMA wait
            warm_ps = psA.tile([128, 512], F32, tag="psA", name="warm_ps")
            for _ in range(10):
                nc.tensor.matmul(
                    warm_ps[:], lhsT=warm_lhs, rhs=warm_rhs,
                    start=True, stop=True,
                )

            # start-critical loads: m g0 leads the sync queue, hstk key
            # blocks follow; m g1..g7 stream on the scalar HW queue with the
            # big phase-B hsbk load queued behind them. The slow gpsimd
            # software queue only carries the tiny bias now; q0/q1 are
            # pinned behind early chains below.
            n_sync = _kb_split(kb)
            nc.sync.dma_start(out=m_sb[:, 0:MG_OFS[1]], in_=mt.ap()[:, 0:MG_OFS[1]])
            for o, w in kb[:n_sync]:
                nc.sync.dma_start(
                    out=hstk_sb[:, HC * o:HC * (o + w)], in_=hstk.ap()[:, HC * o:HC * (o + w)]
                )
            for g in range(1, HC):
                nc.scalar.dma_start(
                    out=m_sb[:, MG_OFS[g]:MG_OFS[g + 1]],
                    in_=mt.ap()[:, MG_OFS[g]:MG_OFS[g + 1]],
                )
            for o, w in kb[n_sync:]:
                nc.scalar.dma_start(
                    out=hstk_sb[:, HC * o:HC * (o + w)], in_=hstk.ap()[:, HC * o:HC * (o + w)]
                )
            nc.gpsimd.dma_start(out=bias_sb[:], in_=mk.ap()[:, :])

            q0 = qcol_pool.tile([128, HC * 512], F16, tag="qcol", name="qcol")
            q1 = qcol_pool.tile([128, HC * 512], F16, tag="qcol", name="qcol")

            def m_lhsT(h, oc):
                base = MG_OFS[oc] + h * 128
                return m_sb[:, base:base + 128]

            order = _phase_a_order(kb)
            # phase-B load release points by cumulative PE progress: hsbk at
            # ~25% (queues on scalar behind m+hstk anyway), q0/q1 (gpsimd
            # software queue) at ~45%/~65% so their transfers run after the
            # phase-A input window yet land well before phase B reads them.
            # Measured failure mode of early release: q0/q1 at t~13-30us ate
    	    # the HBM budget and m g6/g7 crawled at 11-50GB/s (7.4us PE stall).
            total_rows = sum(kb[j][1] for j, _g in order)
            cum_rows = 0.0
            pin_hsbk = pin_q0 = pin_q1 = -1
            for i, (j, _g) in enumerate(order):
                cum_rows += kb[j][1]
                if pin_hsbk < 0 and cum_rows >= 0.25 * total_rows:
                    pin_hsbk = i
                if pin_q0 < 0 and cum_rows >= 0.45 * total_rows:
                    pin_q0 = i
                if pin_q1 < 0 and cum_rows >= 0.65 * total_rows:
                    pin_q1 = i
            for idx, (j, oc) in enumerate(order):
                o, w = kb[j]
                ps = psA.tile([128, 512], F32, tag="psA", name="psa")
                for h in range(HC):
                    nc.tensor.matmul(
                        ps[:, 0:w],
                        lhsT=m_lhsT(h, oc),
                        rhs=hstk_sb[:, HC * o + h * w:HC * o + (h + 1) * w],
                        start=(h == 0),
                        stop=(h == HC - 1),
                    )
                nc.vector.tensor_copy(out=gt[oc][:, o:o + w], in_=ps[:, 0:w])
                # Pin the phase-B loads behind early chains via dummy-copy
                # WAW deps so their transfers stay out of the phase-A HBM
                # bandwidth window (the scheduler hoists dependency-free DMA
                # triggers to the engine-stream front).
                if idx == pin_hsbk:
                    nc.vector.tensor_copy(out=hsbk_sb[:, 0:1], in_=gt[oc][:, o:o + 1])
                    nc.scalar.dma_start(out=hsbk_sb[:], in_=hsbk.ap()[:, :])
                if idx == pin_q0:
                    nc.vector.tensor_copy(out=q0[:, 0:1], in_=gt[oc][:, o:o + 1])
                    nc.gpsimd.dma_start(out=q0[:], in_=hstq.ap()[:, 0:HC * 512])
                if idx == pin_q1:
                    nc.vector.tensor_copy(out=q1[:, 0:1], in_=gt[oc][:, o:o + 1])
                    nc.gpsimd.dma_start(out=q1[:], in_=hstq.ap()[:, HC * 512:2 * HC * 512])

        # ---- Phase B: scores^T -> exp -> attention-value, per 512-wide block
        # of query positions.
        with ExitStack() as pb:
            et_pool = pb.enter_context(tc.tile_pool(name="etp", bufs=1))
            esum_pool = pb.enter_context(tc.tile_pool(name="esump", bufs=2))
            ps_o = pb.enter_context(tc.tile_pool(name="pso", bufs=2, space="PSUM"))
            ps_n = pb.enter_context(tc.tile_pool(name="psn", bufs=1, space="PSUM"))
            out_pool = pb.enter_context(tc.tile_pool(name="outp", bufs=2))
            r_pool = pb.enter_context(tc.tile_pool(name="rp", bufs=2))

            for qb in range(QB):
                if qb == 0:
                    qcol = q0
                elif qb == 1:
                    qcol = q1
                else:
                    qcol = qnext
                if 1 <= qb < QB - 1:
                    # prefetch block qb+1; the pool slot reuse (bufs=2) makes
                    # this DMA wait for block qb-1's last reader, keeping the
                    # transfer out of earlier bandwidth windows
                    qnext = qcol_pool.tile([128, HC * 512], F16, tag="qcol", name="qcol")
                    nc.sync.dma_start(
                        out=qnext[:],
                        in_=hstq.ap()[:, (qb + 1) * HC * 512:(qb + 2) * HC * 512],
                    )
                et = [et_pool.tile([128, 512], F16, tag=f"et{k}", name=f"et{k}") for k in range(nkc)]
                esum = esum_pool.tile([128, 512], F16, tag="esum", name="esum")
                for k in range(nkc):
                    ps = ps_s.tile([128, 512], F32, tag="pss", name="pss")
                    for d in range(HC):
                        nc.tensor.matmul(
                            ps[:],
                            lhsT=gt[d][:, k * 128:(k + 1) * 128],
                            rhs=qcol[:, d * 512:(d + 1) * 512],
                            start=(d == 0),
                            stop=(d == HC - 1),
                        )
                    nc.scalar.activation(
                        out=et[k][:], in_=ps[:],
                        func=mybir.ActivationFunctionType.Exp,
                        scale=1.0 / 32.0,
                        bias=bias_sb[:, k:k + 1],
                    )
                    # denominator partial sums on the (otherwise idle) DVE
                    if k == 1:
                        nc.vector.tensor_tensor(
                            out=esum[:], in0=et[0][:], in1=et[1][:],
                            op=mybir.AluOpType.add,
                        )
                    elif k >= 2:
                        nc.vector.tensor_tensor(
                            out=esum[:], in0=et[k][:], in1=esum[:],
                            op=mybir.AluOpType.add,
                        )
                den_src = esum if nkc >= 2 else et[0]
                r4 = None
                for qs in range(4):
                    po0 = ps_o.tile([128, 512], F32, tag="po0", name="po0")
                    po1 = ps_o.tile([128, 512], F32, tag="po1", name="po1")
                    if qb == QB - 1 and qs == 3:
                        # last group: run the po0 chain to completion first so
                        # half0's scale+store overlaps half1's matmuls,
                        # shortening the serial kernel tail
                        for k in range(nkc):
                            nc.tensor.matmul(
                                po0[:], lhsT=et[k][:, qs * 128:(qs + 1) * 128],
                                rhs=hsbk_sb[:, k * H:k * H + 512],
                                start=(k == 0), stop=(k == nkc - 1),
                            )
                        for k in range(nkc):
                            nc.tensor.matmul(
                                po1[:], lhsT=et[k][:, qs * 128:(qs + 1) * 128],
                                rhs=hsbk_sb[:, k * H + 512:(k + 1) * H],
                                start=(k == 0), stop=(k == nkc - 1),
                            )
                    else:
                        for k in range(nkc):
                            lw = et[k][:, qs * 128:(qs + 1) * 128]
                            st, sp = (k == 0), (k == nkc - 1)
                            nc.tensor.matmul(
                                po0[:], lhsT=lw, rhs=hsbk_sb[:, k * H:k * H + 512],
                                start=st, stop=sp,
                            )
                            nc.tensor.matmul(
                                po1[:], lhsT=lw, rhs=hsbk_sb[:, k * H + 512:(k + 1) * H],
                                start=st, stop=sp,
                            )
                    if qs == 0:
                        # all 4 query-group denominators at once: esum is
                        # ready (last DVE add) well before the first AV chain
                        # ends, so these 4 one-column matmuls cost ~130ns.
                        # one accumulation group: start=True zeroes the whole
                        # 2KB bank (zero-region granularity), so only the
                        # first column-matmul may carry it
                        pn4 = ps_n.tile([128, 4], F32, tag="pn4", name="pn4")
                        for q2 in range(4):
                            nc.tensor.matmul(
                                pn4[:, q2:q2 + 1],
                                lhsT=den_src[:, q2 * 128:(q2 + 1) * 128],
                                rhs=ones_sb[:],
                                start=(q2 == 0), stop=(q2 == 3),
                                skip_group_check=True,
                            )
                        r4 = r_pool.tile([128, 4], F32, tag="r4", name="r4")
                        nc.vector.reciprocal(r4[:], pn4[:])
                    # scale + store: one 512-col half per engine/queue pair
                    ot = out_pool.tile([128, H], F16, tag="ot", name="ot")
                    row = qb * 512 + qs * 128
                    nc.vector.tensor_scalar_mul(
                        out=ot[:, 0:512], in0=po0[:], scalar1=r4[:, qs:qs + 1]
                    )
                    nc.sync.dma_start(out=out.ap()[row:row + 128, 0:512], in_=ot[:, 0:512])
                    if qb == QB - 1 and qs == 3:
                        # final serial tail: po1 is the last matmul chain, so
                        # split its scale+store across both engines/queues
                        # (one 512-col scale is 751ns on scalar; two 256-col
                        # halves run in ~450ns parallel, stores overlap too)
                        nc.vector.tensor_scalar_mul(
                            out=ot[:, 512:768], in0=po1[:, 0:256],
                            scalar1=r4[:, qs:qs + 1],
                        )
                        nc.sync.dma_start(
                            out=out.ap()[row:row + 128, 512:768], in_=ot[:, 512:768]
                        )
                        nc.scalar.mul(
                            out=ot[:, 768:1024], in_=po1[:, 256:512],
                            mul=r4[:, qs:qs + 1],
                        )
                        nc.scalar.dma_start(
                            out=out.ap()[row:row + 128, 768:1024], in_=ot[:, 768:1024]
                        )
                    else:
                        nc.scalar.mul(
                            out=ot[:, 512:1024], in_=po1[:], mul=r4[:, qs:qs + 1]
                        )
                        nc.scalar.dma_start(
                            out=out.ap()[row:row + 128, 512:1024], in_=ot[:, 512:1024]
                        )

    nc.finalize()
    return nc


def prep_inputs(hidden_states, key_padding_mask, Wq_w, Wq_b, Wk_w, Wk_b):
    """Host prep: fold weights, compact masked keys, relay to DMA-flat
    [128, X] layouts. Returns (nkc, in_maps)."""
    hs = np.ascontiguousarray(hidden_states, dtype=np.float32)
    mask = np.asarray(key_padding_mask, dtype=bool)
    wq = np.asarray(Wq_w, dtype=np.float64)
    wk = np.asarray(Wk_w, dtype=np.float64)
    bq = np.asarray(Wq_b, dtype=np.float64)
    m16 = (wk.T @ wq).astype(np.float32).astype(np.float16)     # [h, oc]
    u = (wk.T @ bq).astype(np.float32)                          # [h]
    hsu = hs.reshape(-1, H) @ u                                 # [B*S]
    bias = (hsu.reshape(B, S) / 32.0 - 3.0).astype(np.float32)

    kmax = int((~mask).sum(axis=1).max())
    nkc = max(1, -(-kmax // 128))
    nk = nkc * 128
    kb = _kb_blocks(nk)

    # m relaid: per partition p, column groups g, h-major inside each group
    m3 = m16.reshape(HC, 128, H)                                # [h, p, oc]
    mt_l = np.concatenate(
        [m3[:, :, s:s + w].transpose(1, 0, 2).reshape(128, HC * w) for s, w in MG],
        axis=1,
    )

    in_maps = []
    for b in range(B):
        sel = np.flatnonzero(~mask[b])
        kk = len(sel)
        hs16 = hs[b].astype(np.float16)                         # [s, d]
        hsk = np.zeros((nk, H), np.float16)
        hsk[:kk] = hs16[sel]                                    # compacted keys
        mkb = np.full(nk, -1e30, np.float32)
        mkb[:kk] = bias[b][sel]

        hsTk = np.ascontiguousarray(hsk.T)                      # [d, keys]
        k3 = hsTk.reshape(HC, 128, nk)                          # [h, p, key]
        hstk_l = np.concatenate(
            [k3[:, :, o:o + w].transpose(1, 0, 2).reshape(128, HC * w) for o, w in kb],
            axis=1,
        )
        hsbk_l = hsk.reshape(nkc, 128, H).transpose(1, 0, 2).reshape(128, nkc * H)
        q3 = hs16.T.reshape(HC, 128, S)                         # [h, p, q]
        hstq_l = np.concatenate(
            [q3[:, :, qb * 512:(qb + 1) * 512].transpose(1, 0, 2).reshape(128, HC * 512)
             for qb in range(QB)],
            axis=1,
        )
        in_maps.append({
            "hstq": np.ascontiguousarray(hstq_l),
            "hstk": np.ascontiguousarray(hstk_l),
            "hsbk": np.ascontiguousarray(hsbk_l),
            "mt": mt_l,
            "mk": np.ascontiguousarray(mkb.reshape(nkc, 128).T),
        })
    return nkc, in_maps


def kernel(hidden_states, key_padding_mask, Wq_w, Wq_b, Wk_w, Wk_b):
    nkc, in_maps = prep_inputs(
        hidden_states, key_padding_mask, Wq_w, Wq_b, Wk_w, Wk_b
    )
    nc = _CACHED_NC.get(nkc)
    if nc is None:
        nc = _CACHED_NC[nkc] = build_nc(nkc)

    res = run_bass_kernel_spmd(nc, in_maps, core_ids=list(range(N_CORES)))
    return np.stack(
        [np.asarray(res.results[b]["out"]) for b in range(B)]
    ).astype(np.float32)


# revision 22
# speedup vs baseline: 1.0017x; 1.0017x over previous
"""Trainium2 Bass kernel for AttnNoProjVal.

Per batch element b (one NeuronCore each, B=8), using the identity
  scores = q k^T = hs M hs^T + (hs u) 1^T + 1 (hs v)^T + bk.bq,
  M = Wk^T Wq (host-folded), u = Wk^T bq, v = Wq^T bk:
the v and constant terms are per-QUERY-column offsets, which cancel exactly
in softmax and are dropped; the u term is a per-KEY offset folded into the
exp bias. The kernel computes one fused projection g^T = M^T hs^T, then
  scoresT[kp,qp] = (g^T)[:,kp] . (hsq^T)[:,qp]
  E = exp(scoresT/32 + bias[kp])    bias = (hs u)/32 - 3 + mask (host-prep)
  out[qp,:] = (E^T hsbk) / den,  den via esum = sum_k E[k] on DVE then a
  single [128,1] ones-matmul per 128-query group.

vs the v1 fp16 kernel (trace-driven, NTFF analysis; v1 347us -> 280us
traced at warm clock, run-to-run P0 power-downclock adds ~18%):
- the denominator's 240 one-column matmuls (~32ns NX floor each, ~7.6us of
  PE time) are replaced by DVE chunk-adds (idle engine) + 16 tiny matmuls.
- phase-A start: sync queue carries m g0 then hstk b0..b4, scalar queue m
  g1..g7 then hstk b5/b6; first key blocks shrunk to 64/64/128 wide and the
  (block, oc) chains emitted in a wavefront matching DMA arrival order.
  PE gaps in phase A dropped from ~7.4us (v2: sync's big late hstk blocks +
  early q0/q1 ate HBM while m g6/g7 crawled at 11-50GB/s) to ~1.4us.
- 8 warm-up matmuls on stride-0 broadcasts of the const-pool bf16 1.0
  scalar (memset by the NEFF preamble BEFORE the kernel-entry barrier, so
  the PE starts at ~7.3us with zero in-kernel deps) pre-warm the HAM clock
  gate (cold PE runs at 1.2GHz for the first ~3.4us of activity); real
  chains then start at full rate the moment their DMA data lands.
- scores PSUM pool allocated in the outer scope on banks phase-A never
  touches: allocating it inside phase B serialized its first tile behind
  all 56 phase-A PSUM->SBUF casts (pool-boundary wait, 650ns PE gap).
  Phase-A PSUM shrinks to 5 bufs (still ahead of the cast turnaround).
- out is stored fp16 (2e-2 rel budget, adds ~5e-4) halving store traffic;
  each 512-col half is scaled and stored by a different engine/queue
  (vector+sync / scalar+scalar) and the last group's po0/po1 chains are
  de-interleaved, shortening the serial tail.
- phase-B loads stay off the phase-A HBM window: hsbk queues behind the m
  groups on the scalar queue (pinned at 25% PE progress); q0/q1 go on the
  gpsimd software queue pinned at 45%/65% via dummy-copy WAW deps (the tile
  scheduler hoists dependency-free DMA triggers to the engine-stream front,
  so emission order alone cannot delay them).
- fp8/DoubleRow rejected by measurement: e4m3's 3-bit mantissa on E or V
  alone gives 3.2e-2 max rel err (budget 2e-2; host-simulated), and
  residual-pair schemes cost >= fp16. Matmul moving dim >512 is blocked by
  the one-PSUM-bank rule, so 512-row fp16 streams are the PE floor
  (measured: 216ns/512-row spacing warm = the documented roofline; LDW
  fully hidden). Remaining fixed overhead: ~7us NEFF preamble + ~10us
  runtime semaphore-zero teardown, not kernel-controllable.
"""

import sys

sys.path.insert(0, "/opt/trn_rl_repo")

from contextlib import ExitStack

import numpy as np

import concourse.tile as tile
from concourse import bacc, mybir
from concourse.bass_utils import run_bass_kernel_spmd

B, S, H = 8, 2048, 1024
N_CORES = 8
HC = H // 128   # 8 chunks of the hidden dim
QB = S // 512   # 4 query blocks
F32 = mybir.dt.float32
F16 = mybir.dt.float16

NKC_DEFAULT = 15  # key chunks after mask compaction (padded to 128)

# m in per-oc column blocks, h-major inside each group
MG = [(i * 128, 128) for i in range(HC)]  # (start, width)
MG_OFS = [0]
for _s, _w in MG:
    MG_OFS.append(MG_OFS[-1] + HC * _w)

_CACHED_NC = {}


def _kb_blocks(nk):
    """key-block (offset, width) list for the projection moving dim; small
    leading blocks (64/64/128/256) shrink the first chain's DMA dependency
    so the PE starts as soon as m g0 + 64 keys have landed."""
    kb = []
    o = 0
    for w in (64, 64, 128, 256):
        if o < nk and nk - o >= w:
            kb.append((o, w))
            o += w
    while o < nk:
        w = min(512, nk - o)
        kb.append((o, w))
        o += w
    return kb


def _kb_split(kb):
    """hstk blocks: all but the last two stream on sync (after m g0); the
    last two queue on scalar behind the m groups, so the sync queue's big
    late blocks can't starve the m stream mid-phase (measured: sync ran at
    319GB/s on b4-b6 at t=16-19us while scalar's m g6/g7 crawled at
    11-50GB/s, stalling the PE 7.4us)."""
    n_sync = max(1, len(kb) - 2)
    return n_sync


def _phase_a_order(kb):
    """(block, oc) chain emission order matching expected DMA arrival:
    sync = m g0, hstk b0..b[n-3]; scalar = m g1..g7, hstk b[n-2], b[n-1].
    Times in us relative to sync-queue first data; scalar queue observed
    ~1.4us later."""
    n_sync = _kb_split(kb)
    m_t = [0.0] * HC
    m_t[0] = 1.31  # g0 leads the sync queue
    for g in range(1, HC):
        m_t[g] = 1.4 + 1.31 * g
    k_t = [0.0] * len(kb)
    cum = 1.31  # after m g0 on sync
    for j in range(n_sync):
        cum += (kb[j][1] * 2048) / 190e3
        k_t[j] = cum
    cum = m_t[HC - 1]  # scalar: after the last m group
    for j in range(n_sync, len(kb)):
        cum += (kb[j][1] * 2048) / 190e3
        k_t[j] = cum
    pairs = [(j, g) for j in range(len(kb)) for g in range(HC)]
    pairs.sort(key=lambda p: (max(m_t[p[1]], k_t[p[0]]), p[0], p[1]))
    return pairs


def build_nc(nkc=NKC_DEFAULT):
    nk = nkc * 128
    kb = _kb_blocks(nk)
    nc = bacc.Bacc(None, target_bir_lowering=False)

    # all inputs host-relaid to [128 partitions, X] with contiguous rows
    hstq = nc.dram_tensor("hstq", [128, QB * HC * 512], F16, kind="ExternalInput")
    hstk = nc.dram_tensor("hstk", [128, HC * nk], F16, kind="ExternalInput")
    hsbk = nc.dram_tensor("hsbk", [128, nkc * H], F16, kind="ExternalInput")
    mt = nc.dram_tensor("mt", [128, HC * H], F16, kind="ExternalInput")
    mk = nc.dram_tensor("mk", [128, nkc], F32, kind="ExternalInput")
    out = nc.dram_tensor("out", [S, H], F16, kind="ExternalOutput")

    with tile.TileContext(nc) as tc, ExitStack() as whole:
        singles = whole.enter_context(tc.tile_pool(name="singles", bufs=1))
        gt_pool = whole.enter_context(tc.tile_pool(name="gtp", bufs=1))
        hsbk_pool = whole.enter_context(tc.tile_pool(name="hsbkp", bufs=1))
        qcol_pool = whole.enter_context(tc.tile_pool(name="qcolp", bufs=2))
        # scores PSUM lives in the outer scope on banks psA never touches:
        # allocating it inside phase B would serialize its first tile behind
        # ALL phase-A CASTs (pool-boundary wait, measured 650ns PE gap)
        ps_s = whole.enter_context(tc.tile_pool(name="pss", bufs=3, space="PSUM"))

        bias_sb = singles.tile([128, nkc], F32, tag="bias", name="bias_sb")
        ones_sb = singles.tile([128, 1], F16, tag="ones", name="ones_sb")
        nc.vector.memset(ones_sb[:], 1.0)

        # g^T = M^T hs^T over compacted keys; resident for the whole kernel
        gt = [gt_pool.tile([128, nk], F16, tag=f"gt{d}", name=f"gt{d}") for d in range(HC)]
        hsbk_sb = hsbk_pool.tile([128, nkc * H], F16, tag="hsbk", name="hsbk_sb")

        # ---- Phase A: fused projection g^T into SBUF.
        with ExitStack() as pa:
            wt_pool = pa.enter_context(tc.tile_pool(name="wtp", bufs=1))
            psA = pa.enter_context(tc.tile_pool(name="psA", bufs=5, space="PSUM"))

            m_sb = wt_pool.tile([128, HC * H], F16, tag="m", name="m_sb")
            hstk_sb = wt_pool.tile([128, HC * nk], F16, tag="hstk", name="hstk_sb")

            # HAM pre-warm: the PE clock gate needs ~3.4us of sustained
            # activity to release (1.2 -> full clock). Operands are stride-0
            # broadcasts of the const-pool bf16 1.0 scalar, which the NEFF
            # preamble memsets BEFORE the kernel-entry barrier — so the first
            # LDWEIGHTS has no in-kernel dependency and the PE starts ~1us
            # earlier than any same-kernel memset allows. The product is
            # never read (next pool tile overwrites with start=True).
            warm_lhs = nc.const_aps.tensor(1.0, (128, 128), mybir.dt.bfloat16)
            warm_rhs = nc.const_aps.tensor(1.0, (128, 512), mybir.dt.bfloat16)
            # 22 dummies (~4.7us of warm-rate work): HAM's MID detector fires
            # on a mostly-idle 3.4us window; with fewer dummies the PE idles
            # between dummy-end (~9.5-11.6us) and the sparse first data-gated
            # chains (~12-13us), and the resulting 4/8 re-throttle at ~15us
            # costs ~1.7us against a full backlog (measured: narrow-width
            # spacing p90 = 2x p50). 22 dummies end at first-data time when
            # the clock starts warm (no idle window at all); on a cold start
            # the extra dummy tail (~1.7us) is offset by the avoided
            # re-throttle.
            warm_ps = psA.tile([128, 512], F32, tag="psA", name="warm_ps")
            for _ in range(22):
                nc.tensor.matmul(
                    warm_ps[:], lhsT=warm_lhs, rhs=warm_rhs,
                    start=True, stop=True,
                )

            # start-critical loads: m g0 leads the sync queue, hstk key
            # blocks follow; m g1..g7 stream on the scalar HW queue with the
            # big phase-B hsbk load queued behind them. The slow gpsimd
            # software queue only carries the tiny bias now; q0/q1 are
            # pinned behind early chains below.
            n_sync = _kb_split(kb)
            nc.sync.dma_start(out=m_sb[:, 0:MG_OFS[1]], in_=mt.ap()[:, 0:MG_OFS[1]])
            for o, w in kb[:n_sync]:
                nc.sync.dma_start(
                    out=hstk_sb[:, HC * o:HC * (o + w)], in_=hstk.ap()[:, HC * o:HC * (o + w)]
                )
            for g in range(1, HC):
                nc.scalar.dma_start(
                    out=m_sb[:, MG_OFS[g]:MG_OFS[g + 1]],
                    in_=mt.ap()[:, MG_OFS[g]:MG_OFS[g + 1]],
                )
            for o, w in kb[n_sync:]:
                nc.scalar.dma_start(
                    out=hstk_sb[:, HC * o:HC * (o + w)], in_=hstk.ap()[:, HC * o:HC * (o + w)]
                )
            nc.gpsimd.dma_start(out=bias_sb[:], in_=mk.ap()[:, :])

            q0 = qcol_pool.tile([128, HC * 512], F16, tag="qcol", name="qcol")
            q1 = qcol_pool.tile([128, HC * 512], F16, tag="qcol", name="qcol")

            def m_lhsT(h, oc):
                base = MG_OFS[oc] + h * 128
                return m_sb[:, base:base + 128]

            order = _phase_a_order(kb)
            # phase-B load release points by cumulative PE progress: hsbk at
            # ~25% (queues on scalar behind m+hstk anyway), q0/q1 (gpsimd
            # software queue) at ~45%/~65% so their transfers run after the
            # phase-A input window yet land well before phase B reads them.
            # Measured failure mode of early release: q0/q1 at t~13-30us ate
    	    # the HBM budget and m g6/g7 crawled at 11-50GB/s (7.4us PE stall).
            total_rows = sum(kb[j][1] for j, _g in order)
            cum_rows = 0.0
            pin_hsbk = pin_q0 = pin_q1 = -1
            for i, (j, _g) in enumerate(order):
                cum_rows += kb[j][1]
                if pin_hsbk < 0 and cum_rows >= 0.25 * total_rows:
                    pin_hsbk = i
                if pin_q0 < 0 and cum_rows >= 0.45 * total_rows:
                    pin_q0 = i
                if pin_q1 < 0 and cum_rows >= 0.65 * total_rows:
                    pin_q1 = i
            for idx, (j, oc) in enumerate(order):
                o, w = kb[j]
                ps = psA.tile([128, 512], F32, tag="psA", name="psa")
                for h in range(HC):
                    nc.tensor.matmul(
                        ps[:, 0:w],
                        lhsT=m_lhsT(h, oc),
                        rhs=hstk_sb[:, HC * o + h * w:HC * o + (h + 1) * w],
                        start=(h == 0),
                        stop=(h == HC - 1),
                    )
                nc.vector.tensor_copy(out=gt[oc][:, o:o + w], in_=ps[:, 0:w])
                # Pin the phase-B loads behind early chains via dummy-copy
                # WAW deps so their transfers stay out of the phase-A HBM
                # bandwidth window (the scheduler hoists dependency-free DMA
                # triggers to the engine-stream front).
                if idx == pin_hsbk:
                    nc.vector.tensor_copy(out=hsbk_sb[:, 0:1], in_=gt[oc][:, o:o + 1])
                    nc.scalar.dma_start(out=hsbk_sb[:], in_=hsbk.ap()[:, :])
                if idx == pin_q0:
                    nc.vector.tensor_copy(out=q0[:, 0:1], in_=gt[oc][:, o:o + 1])
                    nc.gpsimd.dma_start(out=q0[:], in_=hstq.ap()[:, 0:HC * 512])
                if idx == pin_q1:
                    nc.vector.tensor_copy(out=q1[:, 0:1], in_=gt[oc][:, o:o + 1])
                    nc.gpsimd.dma_start(out=q1[:], in_=hstq.ap()[:, HC * 512:2 * HC * 512])

        # ---- Phase B: scores^T -> exp -> attention-value, per 512-wide block
        # of query positions.
        with ExitStack() as pb:
            et_pool = pb.enter_context(tc.tile_pool(name="etp", bufs=1))
            esum_pool = pb.enter_context(tc.tile_pool(name="esump", bufs=2))
            ps_o = pb.enter_context(tc.tile_pool(name="pso", bufs=2, space="PSUM"))
            ps_n = pb.enter_context(tc.tile_pool(name="psn", bufs=1, space="PSUM"))
            out_pool = pb.enter_context(tc.tile_pool(name="outp", bufs=2))
            r_pool = pb.enter_context(tc.tile_pool(name="rp", bufs=2))

            for qb in range(QB):
                if qb == 0:
                    qcol = q0
                elif qb == 1:
                    qcol = q1
                else:
                    qcol = qnext
                if 1 <= qb < QB - 1:
                    # prefetch block qb+1; the pool slot reuse (bufs=2) makes
                    # this DMA wait for block qb-1's last reader, keeping the
                    # transfer out of earlier bandwidth windows
                    qnext = qcol_pool.tile([128, HC * 512], F16, tag="qcol", name="qcol")
                    nc.sync.dma_start(
                        out=qnext[:],
                        in_=hstq.ap()[:, (qb + 1) * HC * 512:(qb + 2) * HC * 512],
                    )
                et = [et_pool.tile([128, 512], F16, tag=f"et{k}", name=f"et{k}") for k in range(nkc)]
                esum = esum_pool.tile([128, 512], F16, tag="esum", name="esum")
                for k in range(nkc):
                    ps = ps_s.tile([128, 512], F32, tag="pss", name="pss")
                    for d in range(HC):
                        nc.tensor.matmul(
                            ps[:],
                            lhsT=gt[d][:, k * 128:(k + 1) * 128],
                            rhs=qcol[:, d * 512:(d + 1) * 512],
                            start=(d == 0),
                            stop=(d == HC - 1),
                        )
                    nc.scalar.activation(
                        out=et[k][:], in_=ps[:],
                        func=mybir.ActivationFunctionType.Exp,
                        scale=1.0 / 32.0,
                        bias=bias_sb[:, k:k + 1],
                    )
                    # denominator partial sums on the (otherwise idle) DVE
                    if k == 1:
                        nc.vector.tensor_tensor(
                            out=esum[:], in0=et[0][:], in1=et[1][:],
                            op=mybir.AluOpType.add,
                        )
                    elif k >= 2:
                        nc.vector.tensor_tensor(
                            out=esum[:], in0=et[k][:], in1=esum[:],
                            op=mybir.AluOpType.add,
                        )
                den_src = esum if nkc >= 2 else et[0]
                r4 = None
                for qs in range(4):
                    po0 = ps_o.tile([128, 512], F32, tag="po0", name="po0")
                    po1 = ps_o.tile([128, 512], F32, tag="po1", name="po1")
                    if qb == QB - 1 and qs == 3:
                        # last group: run the po0 chain to completion first so
                        # half0's scale+store overlaps half1's matmuls,
                        # shortening the serial kernel tail
                        for k in range(nkc):
                            nc.tensor.matmul(
                                po0[:], lhsT=et[k][:, qs * 128:(qs + 1) * 128],
                                rhs=hsbk_sb[:, k * H:k * H + 512],
                                start=(k == 0), stop=(k == nkc - 1),
                            )
                        for k in range(nkc):
                            nc.tensor.matmul(
                                po1[:], lhsT=et[k][:, qs * 128:(qs + 1) * 128],
                                rhs=hsbk_sb[:, k * H + 512:(k + 1) * H],
                                start=(k == 0), stop=(k == nkc - 1),
                            )
                    else:
                        for k in range(nkc):
                            lw = et[k][:, qs * 128:(qs + 1) * 128]
                            st, sp = (k == 0), (k == nkc - 1)
                            nc.tensor.matmul(
                                po0[:], lhsT=lw, rhs=hsbk_sb[:, k * H:k * H + 512],
                                start=st, stop=sp,
                            )
                            nc.tensor.matmul(
                                po1[:], lhsT=lw, rhs=hsbk_sb[:, k * H + 512:(k + 1) * H],
                                start=st, stop=sp,
                            )
                    if qs == 0:
                        # all 4 query-group denominators at once: esum is
                        # ready (last DVE add) well before the first AV chain
                        # ends, so these 4 one-column matmuls cost ~130ns.
                        # one accumulation group: start=True zeroes the whole
                        # 2KB bank (zero-region granularity), so only the
                        # first column-matmul may carry it
                        pn4 = ps_n.tile([128, 4], F32, tag="pn4", name="pn4")
                        for q2 in range(4):
                            nc.tensor.matmul(
                                pn4[:, q2:q2 + 1],
                                lhsT=den_src[:, q2 * 128:(q2 + 1) * 128],
                                rhs=ones_sb[:],
                                start=(q2 == 0), stop=(q2 == 3),
                                skip_group_check=True,
                            )
                        r4 = r_pool.tile([128, 4], F32, tag="r4", name="r4")
                        nc.vector.reciprocal(r4[:], pn4[:])
                    # scale + store: one 512-col half per engine/queue pair
                    ot = out_pool.tile([128, H], F16, tag="ot", name="ot")
                    row = qb * 512 + qs * 128
                    nc.vector.tensor_scalar_mul(
                        out=ot[:, 0:512], in0=po0[:], scalar1=r4[:, qs:qs + 1]
                    )
                    nc.sync.dma_start(out=out.ap()[row:row + 128, 0:512], in_=ot[:, 0:512])
                    if qb == QB - 1 and qs == 3:
                        # final serial tail: po1 is the last matmul chain, so
                        # split its scale+store across both engines/queues
                        # (one 512-col scale is 751ns on scalar; two 256-col
                        # halves run in ~450ns parallel, stores overlap too)
                        nc.vector.tensor_scalar_mul(
                            out=ot[:, 512:768], in0=po1[:, 0:256],
                            scalar1=r4[:, qs:qs + 1],
                        )
                        nc.sync.dma_start(
                            out=out.ap()[row:row + 128, 512:768], in_=ot[:, 512:768]
                        )
                        nc.scalar.mul(
                            out=ot[:, 768:1024], in_=po1[:, 256:512],
                            mul=r4[:, qs:qs + 1],
                        )
                        nc.scalar.dma_start(
                            out=out.ap()[row:row + 128, 768:1024], in_=ot[:, 768:1024]
                        )
                    else:
                        nc.scalar.mul(
                            out=ot[:, 512:1024], in_=po1[:], mul=r4[:, qs:qs + 1]
                        )
                        nc.scalar.dma_start(
                            out=out.ap()[row:row + 128, 512:1024], in_=ot[:, 512:1024]
                        )

    nc.finalize()
    return nc


def prep_inputs(hidden_states, key_padding_mask, Wq_w, Wq_b, Wk_w, Wk_b):
    """Host prep: fold weights, compact masked keys, relay to DMA-flat
    [128, X] layouts. Returns (nkc, in_maps)."""
    hs = np.ascontiguousarray(hidden_states, dtype=np.float32)
    mask = np.asarray(key_padding_mask, dtype=bool)
    wq = np.asarray(Wq_w, dtype=np.float64)
    wk = np.asarray(Wk_w, dtype=np.float64)
    bq = np.asarray(Wq_b, dtype=np.float64)
    m16 = (wk.T @ wq).astype(np.float32).astype(np.float16)     # [h, oc]
    u = (wk.T @ bq).astype(np.float32)                          # [h]
    hsu = hs.reshape(-1, H) @ u                                 # [B*S]
    bias = (hsu.reshape(B, S) / 32.0 - 3.0).astype(np.float32)

    kmax = int((~mask).sum(axis=1).max())
    nkc = max(1, -(-kmax // 128))
    nk = nkc * 128
    kb = _kb_blocks(nk)

    # m relaid: per partition p, column groups g, h-major inside each group
    m3 = m16.reshape(HC, 128, H)                                # [h, p, oc]
    mt_l = np.concatenate(
        [m3[:, :, s:s + w].transpose(1, 0, 2).reshape(128, HC * w) for s, w in MG],
        axis=1,
    )

    in_maps = []
    for b in range(B):
        sel = np.flatnonzero(~mask[b])
        kk = len(sel)
        hs16 = hs[b].astype(np.float16)                         # [s, d]
        hsk = np.zeros((nk, H), np.float16)
        hsk[:kk] = hs16[sel]                                    # compacted keys
        mkb = np.full(nk, -1e30, np.float32)
        mkb[:kk] = bias[b][sel]

        hsTk = np.ascontiguousarray(hsk.T)                      # [d, keys]
        k3 = hsTk.reshape(HC, 128, nk)                          # [h, p, key]
        hstk_l = np.concatenate(
            [k3[:, :, o:o + w].transpose(1, 0, 2).reshape(128, HC * w) for o, w in kb],
            axis=1,
        )
        hsbk_l = hsk.reshape(nkc, 128, H).transpose(1, 0, 2).reshape(128, nkc * H)
        q3 = hs16.T.reshape(HC, 128, S)                         # [h, p, q]
        hstq_l = np.concatenate(
            [q3[:, :, qb * 512:(qb + 1) * 512].transpose(1, 0, 2).reshape(128, HC * 512)
             for qb in range(QB)],
            axis=1,
        )
        in_maps.append({
            "hstq": np.ascontiguousarray(hstq_l),
            "hstk": np.ascontiguousarray(hstk_l),
            "hsbk": np.ascontiguousarray(hsbk_l),
            "mt": mt_l,
            "mk": np.ascontiguousarray(mkb.reshape(nkc, 128).T),
        })
    return nkc, in_maps


def kernel(hidden_states, key_padding_mask, Wq_w, Wq_b, Wk_w, Wk_b):
    nkc, in_maps = prep_inputs(
        hidden_states, key_padding_mask, Wq_w, Wq_b, Wk_w, Wk_b
    )
    nc = _CACHED_NC.get(nkc)
    if nc is None:
        nc = _CACHED_NC[nkc] = build_nc(nkc)

    res = run_bass_kernel_spmd(nc, in_maps, core_ids=list(range(N_CORES)))
    return np.stack(
        [np.asarray(res.results[b]["out"]) for b in range(B)]
    ).astype(np.float32)


# revision 23
# speedup vs baseline: 1.0036x; 1.0019x over previous
"""Trainium2 Bass kernel for AttnNoProjVal.

Per batch element b (one NeuronCore each, B=8), using the identity
  scores = q k^T = hs M hs^T + (hs u) 1^T + 1 (hs v)^T + bk.bq,
  M = Wk^T Wq (host-folded), u = Wk^T bq, v = Wq^T bk:
the v and constant terms are per-QUERY-column offsets, which cancel exactly
in softmax and are dropped; the u term is a per-KEY offset folded into the
exp bias. The kernel computes one fused projection g^T = M^T hs^T, then
  scoresT[kp,qp] = (g^T)[:,kp] . (hsq^T)[:,qp]
  E = exp(scoresT/32 + bias[kp])    bias = (hs u)/32 - 3 + mask (host-prep)
  out[qp,:] = (E^T hsbk) / den,  den via esum = sum_k E[k] on DVE then a
  single [128,1] ones-matmul per 128-query group.

vs the v1 fp16 kernel (trace-driven, NTFF analysis; v1 347us -> 280us
traced at warm clock, run-to-run P0 power-downclock adds ~18%):
- the denominator's 240 one-column matmuls (~32ns NX floor each, ~7.6us of
  PE time) are replaced by DVE chunk-adds (idle engine) + 16 tiny matmuls.
- phase-A start: sync queue carries m g0 then hstk b0..b4, scalar queue m
  g1..g7 then hstk b5/b6; first key blocks shrunk to 64/64/128 wide and the
  (block, oc) chains emitted in a wavefront matching DMA arrival order.
  PE gaps in phase A dropped from ~7.4us (v2: sync's big late hstk blocks +
  early q0/q1 ate HBM while m g6/g7 crawled at 11-50GB/s) to ~1.4us.
- 8 warm-up matmuls on stride-0 broadcasts of the const-pool bf16 1.0
  scalar (memset by the NEFF preamble BEFORE the kernel-entry barrier, so
  the PE starts at ~7.3us with zero in-kernel deps) pre-warm the HAM clock
  gate (cold PE runs at 1.2GHz for the first ~3.4us of activity); real
  chains then start at full rate the moment their DMA data lands.
- scores PSUM pool allocated in the outer scope on banks phase-A never
  touches: allocating it inside phase B serialized its first tile behind
  all 56 phase-A PSUM->SBUF casts (pool-boundary wait, 650ns PE gap).
  Phase-A PSUM shrinks to 5 bufs (still ahead of the cast turnaround).
- out is stored fp16 (2e-2 rel budget, adds ~5e-4) halving store traffic;
  each 512-col half is scaled and stored by a different engine/queue
  (vector+sync / scalar+scalar) and the last group's po0/po1 chains are
  de-interleaved, shortening the serial tail.
- phase-B loads stay off the phase-A HBM window: hsbk queues behind the m
  groups on the scalar queue (pinned at 25% PE progress); q0/q1 go on the
  gpsimd software queue pinned at 45%/65% via dummy-copy WAW deps (the tile
  scheduler hoists dependency-free DMA triggers to the engine-stream front,
  so emission order alone cannot delay them).
- fp8/DoubleRow rejected by measurement: e4m3's 3-bit mantissa on E or V
  alone gives 3.2e-2 max rel err (budget 2e-2; host-simulated), and
  residual-pair schemes cost >= fp16. Matmul moving dim >512 is blocked by
  the one-PSUM-bank rule, so 512-row fp16 streams are the PE floor
  (measured: 216ns/512-row spacing warm = the documented roofline; LDW
  fully hidden). Remaining fixed overhead: ~7us NEFF preamble + ~10us
  runtime semaphore-zero teardown, not kernel-controllable.
"""

import sys

sys.path.insert(0, "/opt/trn_rl_repo")

from contextlib import ExitStack

import numpy as np

import concourse.tile as tile
from concourse import bacc, mybir
from concourse.bass_utils import run_bass_kernel_spmd

B, S, H = 8, 2048, 1024
N_CORES = 8
HC = H // 128   # 8 chunks of the hidden dim
QB = S // 512   # 4 query blocks
F32 = mybir.dt.float32
F16 = mybir.dt.float16

NKC_DEFAULT = 15  # key chunks after mask compaction (padded to 128)

# m in per-oc column blocks, h-major inside each group
MG = [(i * 128, 128) for i in range(HC)]  # (start, width)
MG_OFS = [0]
for _s, _w in MG:
    MG_OFS.append(MG_OFS[-1] + HC * _w)

_CACHED_NC = {}


def _kb_blocks(nk):
    """key-block (offset, width) list for the projection moving dim; small
    leading blocks (64/64/128/256) shrink the first chain's DMA dependency
    so the PE starts as soon as m g0 + 64 keys have landed."""
    kb = []
    o = 0
    for w in (64, 64, 128, 256):
        if o < nk and nk - o >= w:
            kb.append((o, w))
            o += w
    while o < nk:
        w = min(512, nk - o)
        kb.append((o, w))
        o += w
    return kb


def _kb_split(kb):
    """hstk blocks: all but the last two stream on sync (after m g0); the
    last two queue on scalar behind the m groups, so the sync queue's big
    late blocks can't starve the m stream mid-phase (measured: sync ran at
    319GB/s on b4-b6 at t=16-19us while scalar's m g6/g7 crawled at
    11-50GB/s, stalling the PE 7.4us)."""
    n_sync = max(1, len(kb) - 2)
    return n_sync


def _phase_a_order(kb):
    """(block, oc) chain emission order matching expected DMA arrival:
    sync = m g0, hstk b0..b[n-3]; scalar = m g1..g7, hstk b[n-2], b[n-1].
    Times in us relative to sync-queue first data; scalar queue observed
    ~1.4us later."""
    n_sync = _kb_split(kb)
    m_t = [0.0] * HC
    m_t[0] = 1.31  # g0 leads the sync queue
    for g in range(1, HC):
        m_t[g] = 1.4 + 1.31 * g
    k_t = [0.0] * len(kb)
    cum = 1.31  # after m g0 on sync
    for j in range(n_sync):
        cum += (kb[j][1] * 2048) / 190e3
        k_t[j] = cum
    cum = m_t[HC - 1]  # scalar: after the last m group
    for j in range(n_sync, len(kb)):
        cum += (kb[j][1] * 2048) / 190e3
        k_t[j] = cum
    pairs = [(j, g) for j in range(len(kb)) for g in range(HC)]
    pairs.sort(key=lambda p: (max(m_t[p[1]], k_t[p[0]]), p[0], p[1]))
    return pairs


def build_nc(nkc=NKC_DEFAULT):
    nk = nkc * 128
    kb = _kb_blocks(nk)
    nc = bacc.Bacc(None, target_bir_lowering=False)

    # all inputs host-relaid to [128 partitions, X] with contiguous rows
    hstq = nc.dram_tensor("hstq", [128, QB * HC * 512], F16, kind="ExternalInput")
    hstk = nc.dram_tensor("hstk", [128, HC * nk], F16, kind="ExternalInput")
    hsbk = nc.dram_tensor("hsbk", [128, nkc * H], F16, kind="ExternalInput")
    mt = nc.dram_tensor("mt", [128, HC * H], F16, kind="ExternalInput")
    mk = nc.dram_tensor("mk", [128, nkc], F32, kind="ExternalInput")
    out = nc.dram_tensor("out", [S, H], F16, kind="ExternalOutput")

    with tile.TileContext(nc) as tc, ExitStack() as whole:
        singles = whole.enter_context(tc.tile_pool(name="singles", bufs=1))
        gt_pool = whole.enter_context(tc.tile_pool(name="gtp", bufs=1))
        hsbk_pool = whole.enter_context(tc.tile_pool(name="hsbkp", bufs=1))
        qcol_pool = whole.enter_context(tc.tile_pool(name="qcolp", bufs=2))
        # scores PSUM lives in the outer scope on banks psA never touches:
        # allocating it inside phase B would serialize its first tile behind
        # ALL phase-A CASTs (pool-boundary wait, measured 650ns PE gap)
        ps_s = whole.enter_context(tc.tile_pool(name="pss", bufs=3, space="PSUM"))

        bias_sb = singles.tile([128, nkc], F32, tag="bias", name="bias_sb")
        ones_sb = singles.tile([128, 1], F16, tag="ones", name="ones_sb")
        nc.vector.memset(ones_sb[:], 1.0)

        # g^T = M^T hs^T over compacted keys; resident for the whole kernel
        gt = [gt_pool.tile([128, nk], F16, tag=f"gt{d}", name=f"gt{d}") for d in range(HC)]
        hsbk_sb = hsbk_pool.tile([128, nkc * H], F16, tag="hsbk", name="hsbk_sb")

        # ---- Phase A: fused projection g^T into SBUF.
        with ExitStack() as pa:
            wt_pool = pa.enter_context(tc.tile_pool(name="wtp", bufs=1))
            psA = pa.enter_context(tc.tile_pool(name="psA", bufs=5, space="PSUM"))

            m_sb = wt_pool.tile([128, HC * H], F16, tag="m", name="m_sb")
            hstk_sb = wt_pool.tile([128, HC * nk], F16, tag="hstk", name="hstk_sb")

            # HAM pre-warm: the PE clock gate needs ~3.4us of sustained
            # activity to release (1.2 -> full clock). Operands are stride-0
            # broadcasts of the const-pool bf16 1.0 scalar, which the NEFF
            # preamble memsets BEFORE the kernel-entry barrier — so the first
            # LDWEIGHTS has no in-kernel dependency and the PE starts ~1us
            # earlier than any same-kernel memset allows. The product is
            # never read (next pool tile overwrites with start=True).
            warm_lhs = nc.const_aps.tensor(1.0, (128, 128), mybir.dt.bfloat16)
            warm_rhs = nc.const_aps.tensor(1.0, (128, 512), mybir.dt.bfloat16)
            # 20 dummies (~4.3us of warm-rate work): HAM's MID detector fires
            # on a mostly-idle 3.4us window; with fewer dummies the PE idles
            # between dummy-end (~9.5-11.6us) and the sparse first data-gated
            # chains (~12-13us), and the resulting 4/8 re-throttle at ~15us
            # costs ~1.7us against a full backlog (measured: narrow-width
            # spacing p90 = 2x p50). dummies end at the chain-dense time (~13us cold) when
            # the clock starts warm (no idle window at all); on a cold start
            # the extra dummy tail (~1.7us) is offset by the avoided
            # re-throttle.
            warm_ps = psA.tile([128, 512], F32, tag="psA", name="warm_ps")
            for _ in range(20):
                nc.tensor.matmul(
                    warm_ps[:], lhsT=warm_lhs, rhs=warm_rhs,
                    start=True, stop=True,
                )

            # start-critical loads: m g0 leads the sync queue, hstk key
            # blocks follow; m g1..g7 stream on the scalar HW queue with the
            # big phase-B hsbk load queued behind them. The slow gpsimd
            # software queue only carries the tiny bias now; q0/q1 are
            # pinned behind early chains below.
            n_sync = _kb_split(kb)
            nc.sync.dma_start(out=m_sb[:, 0:MG_OFS[1]], in_=mt.ap()[:, 0:MG_OFS[1]])
            for o, w in kb[:n_sync]:
                nc.sync.dma_start(
                    out=hstk_sb[:, HC * o:HC * (o + w)], in_=hstk.ap()[:, HC * o:HC * (o + w)]
                )
            for g in range(1, HC):
                nc.scalar.dma_start(
                    out=m_sb[:, MG_OFS[g]:MG_OFS[g + 1]],
                    in_=mt.ap()[:, MG_OFS[g]:MG_OFS[g + 1]],
                )
            for o, w in kb[n_sync:]:
                nc.scalar.dma_start(
                    out=hstk_sb[:, HC * o:HC * (o + w)], in_=hstk.ap()[:, HC * o:HC * (o + w)]
                )
            nc.gpsimd.dma_start(out=bias_sb[:], in_=mk.ap()[:, :])

            q0 = qcol_pool.tile([128, HC * 512], F16, tag="qcol", name="qcol")
            q1 = qcol_pool.tile([128, HC * 512], F16, tag="qcol", name="qcol")

            def m_lhsT(h, oc):
                base = MG_OFS[oc] + h * 128
                return m_sb[:, base:base + 128]

            order = _phase_a_order(kb)
            # phase-B load release points by cumulative PE progress: hsbk at
            # ~25% (queues on scalar behind m+hstk anyway), q0/q1 (gpsimd
            # software queue) at ~45%/~65% so their transfers run after the
            # phase-A input window yet land well before phase B reads them.
            # Measured failure mode of early release: q0/q1 at t~13-30us ate
    	    # the HBM budget and m g6/g7 crawled at 11-50GB/s (7.4us PE stall).
            total_rows = sum(kb[j][1] for j, _g in order)
            cum_rows = 0.0
            pin_hsbk = pin_q0 = pin_q1 = -1
            for i, (j, _g) in enumerate(order):
                cum_rows += kb[j][1]
                if pin_hsbk < 0 and cum_rows >= 0.25 * total_rows:
                    pin_hsbk = i
                if pin_q0 < 0 and cum_rows >= 0.45 * total_rows:
                    pin_q0 = i
                if pin_q1 < 0 and cum_rows >= 0.65 * total_rows:
                    pin_q1 = i
            for idx, (j, oc) in enumerate(order):
                o, w = kb[j]
                ps = psA.tile([128, 512], F32, tag="psA", name="psa")
                for h in range(HC):
                    nc.tensor.matmul(
                        ps[:, 0:w],
                        lhsT=m_lhsT(h, oc),
                        rhs=hstk_sb[:, HC * o + h * w:HC * o + (h + 1) * w],
                        start=(h == 0),
                        stop=(h == HC - 1),
                    )
                nc.vector.tensor_copy(out=gt[oc][:, o:o + w], in_=ps[:, 0:w])
                # Pin the phase-B loads behind early chains via dummy-copy
                # WAW deps so their transfers stay out of the phase-A HBM
                # bandwidth window (the scheduler hoists dependency-free DMA
                # triggers to the engine-stream front).
                if idx == pin_hsbk:
                    nc.vector.tensor_copy(out=hsbk_sb[:, 0:1], in_=gt[oc][:, o:o + 1])
                    nc.scalar.dma_start(out=hsbk_sb[:], in_=hsbk.ap()[:, :])
                if idx == pin_q0:
                    nc.vector.tensor_copy(out=q0[:, 0:1], in_=gt[oc][:, o:o + 1])
                    nc.gpsimd.dma_start(out=q0[:], in_=hstq.ap()[:, 0:HC * 512])
                if idx == pin_q1:
                    nc.vector.tensor_copy(out=q1[:, 0:1], in_=gt[oc][:, o:o + 1])
                    nc.gpsimd.dma_start(out=q1[:], in_=hstq.ap()[:, HC * 512:2 * HC * 512])

        # ---- Phase B: scores^T -> exp -> attention-value, per 512-wide block
        # of query positions.
        with ExitStack() as pb:
            et_pool = pb.enter_context(tc.tile_pool(name="etp", bufs=1))
            esum_pool = pb.enter_context(tc.tile_pool(name="esump", bufs=2))
            ps_o = pb.enter_context(tc.tile_pool(name="pso", bufs=2, space="PSUM"))
            ps_n = pb.enter_context(tc.tile_pool(name="psn", bufs=1, space="PSUM"))
            out_pool = pb.enter_context(tc.tile_pool(name="outp", bufs=2))
            r_pool = pb.enter_context(tc.tile_pool(name="rp", bufs=2))

            for qb in range(QB):
                if qb == 0:
                    qcol = q0
                elif qb == 1:
                    qcol = q1
                else:
                    qcol = qnext
                if 1 <= qb < QB - 1:
                    # prefetch block qb+1; the pool slot reuse (bufs=2) makes
                    # this DMA wait for block qb-1's last reader, keeping the
                    # transfer out of earlier bandwidth windows
                    qnext = qcol_pool.tile([128, HC * 512], F16, tag="qcol", name="qcol")
                    nc.sync.dma_start(
                        out=qnext[:],
                        in_=hstq.ap()[:, (qb + 1) * HC * 512:(qb + 2) * HC * 512],
                    )
                et = [et_pool.tile([128, 512], F16, tag=f"et{k}", name=f"et{k}") for k in range(nkc)]
                esum = esum_pool.tile([128, 512], F16, tag="esum", name="esum")
                for k in range(nkc):
                    ps = ps_s.tile([128, 512], F32, tag="pss", name="pss")
                    for d in range(HC):
                        nc.tensor.matmul(
                            ps[:],
                            lhsT=gt[d][:, k * 128:(k + 1) * 128],
                            rhs=qcol[:, d * 512:(d + 1) * 512],
                            start=(d == 0),
                            stop=(d == HC - 1),
                        )
                    nc.scalar.activation(
                        out=et[k][:], in_=ps[:],
                        func=mybir.ActivationFunctionType.Exp,
                        scale=1.0 / 32.0,
                        bias=bias_sb[:, k:k + 1],
                    )
                    # denominator partial sums on the (otherwise idle) DVE
                    if k == 1:
                        nc.vector.tensor_tensor(
                            out=esum[:], in0=et[0][:], in1=et[1][:],
                            op=mybir.AluOpType.add,
                        )
                    elif k >= 2:
                        nc.vector.tensor_tensor(
                            out=esum[:], in0=et[k][:], in1=esum[:],
                            op=mybir.AluOpType.add,
                        )
                den_src = esum if nkc >= 2 else et[0]
                r4 = None
                for qs in range(4):
                    po0 = ps_o.tile([128, 512], F32, tag="po0", name="po0")
                    po1 = ps_o.tile([128, 512], F32, tag="po1", name="po1")
                    if qb == QB - 1 and qs == 3:
                        # last group: run the po0 chain to completion first so
                        # half0's scale+store overlaps half1's matmuls,
                        # shortening the serial kernel tail
                        for k in range(nkc):
                            nc.tensor.matmul(
                                po0[:], lhsT=et[k][:, qs * 128:(qs + 1) * 128],
                                rhs=hsbk_sb[:, k * H:k * H + 512],
                                start=(k == 0), stop=(k == nkc - 1),
                            )
                        for k in range(nkc):
                            nc.tensor.matmul(
                                po1[:], lhsT=et[k][:, qs * 128:(qs + 1) * 128],
                                rhs=hsbk_sb[:, k * H + 512:(k + 1) * H],
                                start=(k == 0), stop=(k == nkc - 1),
                            )
                    else:
                        for k in range(nkc):
                            lw = et[k][:, qs * 128:(qs + 1) * 128]
                            st, sp = (k == 0), (k == nkc - 1)
                            nc.tensor.matmul(
                                po0[:], lhsT=lw, rhs=hsbk_sb[:, k * H:k * H + 512],
                                start=st, stop=sp,
                            )
                            nc.tensor.matmul(
                                po1[:], lhsT=lw, rhs=hsbk_sb[:, k * H + 512:(k + 1) * H],
                                start=st, stop=sp,
                            )
                    if qs == 0:
                        # all 4 query-group denominators at once: esum is
                        # ready (last DVE add) well before the first AV chain
                        # ends, so these 4 one-column matmuls cost ~130ns.
                        # one accumulation group: start=True zeroes the whole
                        # 2KB bank (zero-region granularity), so only the
                        # first column-matmul may carry it
                        pn4 = ps_n.tile([128, 4], F32, tag="pn4", name="pn4")
                        for q2 in range(4):
                            nc.tensor.matmul(
                                pn4[:, q2:q2 + 1],
                                lhsT=den_src[:, q2 * 128:(q2 + 1) * 128],
                                rhs=ones_sb[:],
                                start=(q2 == 0), stop=(q2 == 3),
                                skip_group_check=True,
                            )
                        r4 = r_pool.tile([128, 4], F32, tag="r4", name="r4")
                        nc.vector.reciprocal(r4[:], pn4[:])
                    # scale + store: one 512-col half per engine/queue pair
                    ot = out_pool.tile([128, H], F16, tag="ot", name="ot")
                    row = qb * 512 + qs * 128
                    nc.vector.tensor_scalar_mul(
                        out=ot[:, 0:512], in0=po0[:], scalar1=r4[:, qs:qs + 1]
                    )
                    nc.sync.dma_start(out=out.ap()[row:row + 128, 0:512], in_=ot[:, 0:512])
                    if qb == QB - 1 and qs == 3:
                        # final serial tail: po1 is the last matmul chain, so
                        # split its scale+store across both engines/queues
                        # (one 512-col scale is 751ns on scalar; two 256-col
                        # halves run in ~450ns parallel, stores overlap too)
                        nc.vector.tensor_scalar_mul(
                            out=ot[:, 512:768], in0=po1[:, 0:256],
                            scalar1=r4[:, qs:qs + 1],
                        )
                        nc.sync.dma_start(
                            out=out.ap()[row:row + 128, 512:768], in_=ot[:, 512:768]
                        )
                        nc.scalar.mul(
                            out=ot[:, 768:1024], in_=po1[:, 256:512],
                            mul=r4[:, qs:qs + 1],
                        )
                        nc.scalar.dma_start(
                            out=out.ap()[row:row + 128, 768:1024], in_=ot[:, 768:1024]
                        )
                    else:
                        nc.scalar.mul(
                            out=ot[:, 512:1024], in_=po1[:], mul=r4[:, qs:qs + 1]
                        )
                        nc.scalar.dma_start(
                            out=out.ap()[row:row + 128, 512:1024], in_=ot[:, 512:1024]
                        )

    nc.finalize()
    return nc


def prep_inputs(hidden_states, key_padding_mask, Wq_w, Wq_b, Wk_w, Wk_b):
    """Host prep: fold weights, compact masked keys, relay to DMA-flat
    [128, X] layouts. Returns (nkc, in_maps)."""
    hs = np.ascontiguousarray(hidden_states, dtype=np.float32)
    mask = np.asarray(key_padding_mask, dtype=bool)
    wq = np.asarray(Wq_w, dtype=np.float64)
    wk = np.asarray(Wk_w, dtype=np.float64)
    bq = np.asarray(Wq_b, dtype=np.float64)
    m16 = (wk.T @ wq).astype(np.float32).astype(np.float16)     # [h, oc]
    u = (wk.T @ bq).astype(np.float32)                          # [h]
    hsu = hs.reshape(-1, H) @ u                                 # [B*S]
    bias = (hsu.reshape(B, S) / 32.0 - 3.0).astype(np.float32)

    kmax = int((~mask).sum(axis=1).max())
    nkc = max(1, -(-kmax // 128))
    nk = nkc * 128
    kb = _kb_blocks(nk)

    # m relaid: per partition p, column groups g, h-major inside each group
    m3 = m16.reshape(HC, 128, H)                                # [h, p, oc]
    mt_l = np.concatenate(
        [m3[:, :, s:s + w].transpose(1, 0, 2).reshape(128, HC * w) for s, w in MG],
        axis=1,
    )

    in_maps = []
    for b in range(B):
        sel = np.flatnonzero(~mask[b])
        kk = len(sel)
        hs16 = hs[b].astype(np.float16)                         # [s, d]
        hsk = np.zeros((nk, H), np.float16)
        hsk[:kk] = hs16[sel]                                    # compacted keys
        mkb = np.full(nk, -1e30, np.float32)
        mkb[:kk] = bias[b][sel]

        hsTk = np.ascontiguousarray(hsk.T)                      # [d, keys]
        k3 = hsTk.reshape(HC, 128, nk)                          # [h, p, key]
        hstk_l = np.concatenate(
            [k3[:, :, o:o + w].transpose(1, 0, 2).reshape(128, HC * w) for o, w in kb],
            axis=1,
        )
        hsbk_l = hsk.reshape(nkc, 128, H).transpose(1, 0, 2).reshape(128, nkc * H)
        q3 = hs16.T.reshape(HC, 128, S)                         # [h, p, q]
        hstq_l = np.concatenate(
            [q3[:, :, qb * 512:(qb + 1) * 512].transpose(1, 0, 2).reshape(128, HC * 512)
             for qb in range(QB)],
            axis=1,
        )
        in_maps.append({
            "hstq": np.ascontiguousarray(hstq_l),
            "hstk": np.ascontiguousarray(hstk_l),
            "hsbk": np.ascontiguousarray(hsbk_l),
            "mt": mt_l,
            "mk": np.ascontiguousarray(mkb.reshape(nkc, 128).T),
        })
    return nkc, in_maps


def kernel(hidden_states, key_padding_mask, Wq_w, Wq_b, Wk_w, Wk_b):
    nkc, in_maps = prep_inputs(
        hidden_states, key_padding_mask, Wq_w, Wq_b, Wk_w, Wk_b
    )
    nc = _CACHED_NC.get(nkc)
    if nc is None:
        nc = _CACHED_NC[nkc] = build_nc(nkc)

    res = run_bass_kernel_spmd(nc, in_maps, core_ids=list(range(N_CORES)))
    return np.stack(
        [np.asarray(res.results[b]["out"]) for b in range(B)]
    ).astype(np.float32)
